# revision 23
# baseline (speedup 1.0000x reference)
"""EuclideanCodebook (VQ) Trainium2 Bass kernel.

Strategy (data-parallel over 8 NeuronCores, x sharded on tokens):
  per 128-token tile:
    - scores s[t,k] = 2*x.e_k  via fp16 hi/lo split matmuls (3 passes, exact to
      ~2^-22) accumulated in PSUM fp32;
    - fused DVE tensor_tensor_reduce: adj = s - |e_k|^2, m[t] = max_k adj
    - onehot = (adj >= m) on DVE (exact fp32 compare), fp16
    - ind[t] = sum_k onehot*iota  (scalar_tensor_tensor with accum)
    - quantize rows gathered from embed DRAM via indirect DMA
    - segment sums: onehot.T @ [x_h] accumulated in PSUM across all tiles
      (8 k-chunks) + count column via onehot.T @ ones
  epilogue: ReduceScatter(sum) of [1024,257] partials over 8 cores; each core
  EMA-updates its 128-row shard of cluster_size/embed_avg/embed.
Host side only shards/concats and reorders embed_ind.
"""
import numpy as np
from contextlib import ExitStack

import concourse.bass as bass
import concourse.bacc as bacc
import concourse.mybir as mybir
import concourse.tile as tile
from concourse.bass_utils import run_bass_kernel_spmd

dt = mybir.dt
F32 = dt.float32
F16 = dt.float16
I32 = dt.int32
Alu = mybir.AluOpType
Act = mybir.ActivationFunctionType

# problem shapes (hardcoded per contract)
B, T, D, K = 8, 8192, 256, 1024
N_CORES = 8
NT = B * T // N_CORES          # tokens per core (8192)
NTILES = NT // 128             # 64
KC = K // 128                  # 8 k-chunks
KSH = K // N_CORES             # 128 codes per core for EMA shard
DECAY, EPS = 0.8, 1e-5


def build_kernel(n_cores=N_CORES, ntiles=NTILES, use_cc=False):
    nt = ntiles * 128
    ksh = K // n_cores
    nc = bacc.Bacc("TRN2", target_bir_lowering=False, debug=False,
                   num_devices=n_cores)

    x_d = nc.dram_tensor("x_sh", [nt, D], F32, kind="ExternalInput")
    e_d = nc.dram_tensor("embed", [K, D], F32, kind="ExternalInput")
    if use_cc:
        cs_d = nc.dram_tensor("cs_sh", [ksh], F32, kind="ExternalInput")
        av_d = nc.dram_tensor("av_sh", [ksh, D], F32, kind="ExternalInput")

    q_d = nc.dram_tensor("quant_sh", [nt, D], F32, kind="ExternalOutput")
    ind_d = nc.dram_tensor("ind_sh", [128, ntiles], I32, kind="ExternalOutput")
    if use_cc:
        csn_d = nc.dram_tensor("cs_new_sh", [ksh], F32, kind="ExternalOutput")
        avn_d = nc.dram_tensor("av_new_sh", [ksh, D], F32,
                               kind="ExternalOutput")
        emn_d = nc.dram_tensor("em_new_sh", [ksh, D], F32,
                               kind="ExternalOutput")
    else:
        part_d = nc.dram_tensor("part_sh", [K, 257], F32,
                                kind="ExternalOutput")

    with tile.TileContext(nc) as tc, ExitStack() as ctx:
        cpool = ctx.enter_context(tc.tile_pool(name="consts", bufs=1))
        spool = ctx.enter_context(tc.tile_pool(name="setup", bufs=1))
        xpool = ctx.enter_context(tc.tile_pool(name="x", bufs=3))
        wpool = ctx.enter_context(tc.tile_pool(name="work", bufs=2))
        qpool = ctx.enter_context(tc.tile_pool(name="q", bufs=3))
        opool = ctx.enter_context(tc.tile_pool(name="out1", bufs=1))
        ps_s = ctx.enter_context(tc.tile_pool(name="pscore", bufs=3,
                                              space="PSUM"))
        ps_a = ctx.enter_context(tc.tile_pool(name="pacc", bufs=1,
                                              space="PSUM"))
        dpool = ctx.enter_context(tc.tile_pool(name="dram", bufs=1,
                                               space="DRAM"))

        # ---------------- constants / setup ----------------
        embT_h0 = cpool.tile([128, K], F16)   # d 0:128, k, fp16 hi of 2*e^T
        embT_h1 = cpool.tile([128, K], F16)   # d 128:256
        embT_l0 = cpool.tile([128, K], F16)
        embT_l1 = cpool.tile([128, K], F16)
        e2row = cpool.tile([1, K], F32)
        e2full = cpool.tile([128, K], F32)
        iota_row = cpool.tile([1, K], F16)
        iotafull = cpool.tile([128, K], F16)
        ones_r32 = cpool.tile([1, 128], F32)
        ones_r16 = cpool.tile([1, 128], F16)
        ident32 = cpool.tile([128, 128], F32)
        ident16 = cpool.tile([128, 128], F16)
        ones_c32 = cpool.tile([128, 1], F32)
        ones_c16 = cpool.tile([128, 1], F16)
        indacc = opool.tile([128, ntiles], I32)

        # persistent PSUM: scatter accumulator (4 banks) + counts (1 bank)
        pscat = ps_a.tile([128, 2048], F32)
        pcnt = ps_a.tile([128, 512], F32)

        ones128 = spool.tile([128, 128], F32)
        nc.vector.memset(ones128[:], 1.0)
        nc.gpsimd.affine_select(ident32[:], ones128[:], pattern=[[-1, 128]],
                                base=0, channel_multiplier=1,
                                compare_op=Alu.is_equal, fill=0.0)
        nc.vector.tensor_copy(ident16[:], ident32[:])
        nc.vector.memset(ones_c32[:], 1.0)
        nc.vector.memset(ones_c16[:], 1.0)

        iota_i = spool.tile([1, K], I32)
        nc.gpsimd.iota(iota_i[:], pattern=[[1, K]], base=0,
                       channel_multiplier=0)
        nc.vector.tensor_copy(iota_row[:], iota_i[:])
        nc.vector.memset(ones_r32[:], 1.0)
        nc.vector.memset(ones_r16[:], 1.0)
        # broadcast iota to all partitions: ones[1,128].T @ iota_row
        for h in range(2):
            ks = slice(h * 512, (h + 1) * 512)
            pb = ps_s.tile([128, 512], F32, name="pb", tag="ps")
            nc.tensor.matmul(pb[:], ones_r16[:], iota_row[0:1, ks])
            nc.vector.tensor_copy(iotafull[:, ks], pb[:])

        # embed transpose + split + e2
        et = spool.tile([128, D], F32)
        t2 = spool.tile([128, D], F32)
        sq = spool.tile([128, 128], F32)
        for c in range(KC):
            et_ = spool.tile([128, D], F32, name=f"et{c}", tag="et")
            nc.sync.dma_start(et_[:], e_d[c * 128:(c + 1) * 128, :])
            # transpose both d-halves into pcnt[:, 0:256] (exact fp32 e^T);
            # single psum group per chunk (2nd start would re-zero the bank)
            nc.tensor.matmul(pcnt[:, 0:128], et_[:, 0:128], ident32[:],
                             is_transpose=True, start=True, stop=False)
            nc.tensor.matmul(pcnt[:, 128:256], et_[:, 128:256], ident32[:],
                             is_transpose=True, start=False, stop=True)
            t2_ = spool.tile([128, D], F32, name=f"t2{c}", tag="t2")
            nc.scalar.mul(t2_[:], pcnt[:, 0:256], 2.0)
            kk = slice(c * 128, (c + 1) * 128)
            nc.scalar.copy(embT_h0[:, kk], t2_[:, 0:128])
            nc.scalar.copy(embT_h1[:, kk], t2_[:, 128:256])
            nc.vector.tensor_tensor(out=embT_l0[:, kk], in0=t2_[:, 0:128],
                                    in1=embT_h0[:, kk], op=Alu.subtract)
            nc.vector.tensor_tensor(out=embT_l1[:, kk], in0=t2_[:, 128:256],
                                    in1=embT_h1[:, kk], op=Alu.subtract)
            # e2 contribution: sum_d e^2 = ones.T @ (eT*eT) ; eT in pcnt.
            # one psum group per 512-wide bank (chunks 4c..4c+3)
            sq0 = spool.tile([128, 128], F32, name=f"sq0{c}", tag="sq0")
            sq1 = spool.tile([128, 128], F32, name=f"sq1{c}", tag="sq1")
            nc.scalar.square(sq0[:], pcnt[:, 0:128])
            nc.scalar.square(sq1[:], pcnt[:, 128:256])
            nc.tensor.matmul(pscat[0:1, kk], ones_c32[:], sq0[:],
                             start=(c % 4 == 0), stop=False)
            nc.tensor.matmul(pscat[0:1, kk], ones_c32[:], sq1[:],
                             start=False, stop=(c % 4 == 3))
        nc.vector.tensor_copy(e2row[0:1, :], pscat[0:1, 0:K])
        # broadcast e2 to all partitions (fp32 matmul, 512-wide chunks)
        for h in range(2):
            ks = slice(h * 512, (h + 1) * 512)
            pb2 = ps_s.tile([128, 512], F32, name="pb2", tag="ps")
            nc.tensor.matmul(pb2[:], ones_r32[:], e2row[0:1, ks])
            nc.vector.tensor_copy(e2full[:, ks], pb2[:])

        # ---------------- main loop ----------------
        for t in range(ntiles):
            rows = slice(t * 128, (t + 1) * 128)
            xt = xpool.tile([128, D], F32, name="xt", tag="xt")
            nc.sync.dma_start(xt[:], x_d[rows, :])
            # fp16 split
            xaug = xpool.tile([128, 258], F16, name="xaug", tag="xaug")
            nc.scalar.copy(xaug[:, 0:256], xt[:])
            nc.vector.memset(xaug[:, 256:258], 1.0)
            xl = xpool.tile([128, D], F16, name="xl", tag="xl")
            nc.vector.tensor_tensor(out=xl[:], in0=xt[:], in1=xaug[:, 0:256],
                                    op=Alu.subtract)
            # transposes (fp16) into a rotating score-pool bank viewed as f16:
            # [0:256]f16 = xh^T, [256:512]f16 = xl^T. One psum group of 4.
            pxt = ps_s.tile([128, 512], F32, name="pxt", tag="ps")
            pf16 = pxt[:].bitcast(F16)
            nc.tensor.matmul(pf16[:, 0:128], xaug[:, 0:128], ident16[:],
                             is_transpose=True, start=True, stop=False)
            nc.tensor.matmul(pf16[:, 128:256], xaug[:, 128:256], ident16[:],
                             is_transpose=True, start=False, stop=False)
            nc.tensor.matmul(pf16[:, 256:384], xl[:, 0:128], ident16[:],
                             is_transpose=True, start=False, stop=False)
            nc.tensor.matmul(pf16[:, 384:512], xl[:, 128:256], ident16[:],
                             is_transpose=True, start=False, stop=True)
            xth = xpool.tile([128, D], F16, name="xth", tag="xth")
            xtl = xpool.tile([128, D], F16, name="xtl", tag="xtl")
            nc.scalar.copy(xth[:], pf16[:, 0:256])
            nc.scalar.copy(xtl[:], pf16[:, 256:512])

            adj = wpool.tile([128, K], F32, name="adj", tag="adj")
            m01 = wpool.tile([128, 2], F32, name="m01", tag="m01")
            # 12 accumulating matmuls ordered for stationary-weight reuse:
            # each weight (xth/xtl d-chunk) serves its rhs over both k-halves
            pss = [ps_s.tile([128, 512], F32, name=f"ps{h}", tag="ps")
                   for h in range(2)]
            plan = [(xth, 0, embT_h0), (xth, 1, embT_h1),
                    (xth, 0, embT_l0), (xth, 1, embT_l1),
                    (xtl, 0, embT_h0), (xtl, 1, embT_h1)]
            nwr = [0, 0]
            for w, dc, rhs in plan:
                for h in range(2):
                    ks = slice(h * 512, (h + 1) * 512)
                    nc.tensor.matmul(pss[h][:], w[:, dc * 128:(dc + 1) * 128],
                                     rhs[:, ks],
                                     start=(nwr[h] == 0),
                                     stop=(nwr[h] == len(plan) - 1))
                    nwr[h] += 1
            for h in range(2):
                ks = slice(h * 512, (h + 1) * 512)
                nc.vector.tensor_tensor(out=adj[:, ks], in0=pss[h][:],
                                        in1=e2full[:, ks], op=Alu.subtract)
            for h in range(2):
                ks = slice(h * 512, (h + 1) * 512)
                nc.vector.tensor_reduce(out=m01[:, h:h + 1], in_=adj[:, ks],
                                        axis=mybir.AxisListType.X, op=Alu.max)
            m = wpool.tile([128, 1], F32, name="m", tag="m")
            nc.vector.tensor_tensor(out=m[:], in0=m01[:, 0:1],
                                    in1=m01[:, 1:2], op=Alu.max)
            oh = wpool.tile([128, K], F16, name="oh", tag="oh")
            nc.gpsimd.tensor_scalar(out=oh[:], in0=adj[:], scalar1=m[:],
                                    scalar2=None, op0=Alu.is_ge)
            # index extraction
            indf = wpool.tile([128, 1], F32, name="indf", tag="indf")
            ohs = wpool.tile([128, K], F16, name="ohs", tag="ohs")
            nc.vector.scalar_tensor_tensor(
                out=ohs[:], in0=oh[:], scalar=0.5,
                in1=iotafull[:],
                op0=Alu.is_ge, op1=Alu.mult, accum_out=indf[:])
            indi = wpool.tile([128, 1], I32, name="indi", tag="indi")
            nc.vector.tensor_copy(indi[:], indf[:])
            nc.vector.tensor_copy(indacc[:, t:t + 1], indi[:])
            # quantize gather: embed rows by index (DRAM -> SBUF -> DRAM)
            q = qpool.tile([128, D], F32, name="q", tag="q")
            nc.gpsimd.indirect_dma_start(
                out=q[:], out_offset=None, in_=e_d[:],
                in_offset=bass.IndirectOffsetOnAxis(ap=indi[:], axis=0))
            nc.sync.dma_start(q_d[rows, :], q[:])
            # scatter: segment sums (8 k-chunks, 2 chunks share a psum bank
            # -> one group per bank) + counts (all 8 cols in one bank/group)
            for c in range(KC):
                kk = slice(c * 128, (c + 1) * 128)
                nc.tensor.matmul(pscat[:, c * 256:(c + 1) * 256],
                                 oh[:, kk], xaug[:, 0:256],
                                 start=(t == 0 and c % 2 == 0),
                                 stop=(t == ntiles - 1 and c % 2 == 1))
                nc.tensor.matmul(pcnt[:, c:c + 1],
                                 oh[:, kk], xaug[:, 256:257],
                                 start=(t == 0 and c == 0),
                                 stop=(t == ntiles - 1 and c == KC - 1))

        nc.sync.dma_start(ind_d[:], indacc[:])

        # ---------------- epilogue: reduce + EMA ----------------
        if not use_cc:
            # flush per-core partial sums; host reduces across cores
            for c in range(KC):
                st = wpool.tile([128, 257], F32, name="st", tag="st")
                nc.scalar.copy(st[:, 0:256], pscat[:, c * 256:(c + 1) * 256])
                nc.vector.tensor_copy(st[:, 256:257], pcnt[:, c:c + 1])
                nc.sync.dma_start(part_d[c * 128:(c + 1) * 128, :], st[:])
        else:
            rs_in = dpool.tile([K, 257], F32)
            rs_out = dpool.tile([K // n_cores, 257], F32)
            for c in range(KC):
                st = wpool.tile([128, 257], F32, name="st", tag="st")
                nc.scalar.copy(st[:, 0:256], pscat[:, c * 256:(c + 1) * 256])
                nc.vector.tensor_copy(st[:, 256:257], pcnt[:, c:c + 1])
                nc.sync.dma_start(rs_in[c * 128:(c + 1) * 128, :], st[:])
            nc.gpsimd.collective_compute(
                "ReduceScatter", Alu.add,
                replica_groups=[list(range(n_cores))],
                ins=[rs_in.opt()], outs=[rs_out.opt()])

            rsb = opool.tile([128, 257], F32)
            nc.sync.dma_start(rsb[0:ksh, :], rs_out[:])
            cs = opool.tile([128, 1], F32)
            av = opool.tile([128, D], F32)
            nc.sync.dma_start(cs[0:ksh, 0:1],
                              cs_d[:].rearrange("(a b) -> a b", b=1))
            nc.sync.dma_start(av[0:ksh, :], av_d[:])
            csn = opool.tile([128, 1], F32)
            avn = opool.tile([128, D], F32)
            emn = opool.tile([128, D], F32)
            # new = (old*4 + seg) * 0.2  == old*0.8 + seg*0.2
            nc.vector.scalar_tensor_tensor(out=csn[0:ksh, :], in0=cs[0:ksh, :],
                                           scalar=4.0, in1=rsb[0:ksh, 256:257],
                                           op0=Alu.mult, op1=Alu.add)
            nc.vector.tensor_scalar_mul(csn[0:ksh, :], csn[0:ksh, :], 0.2)
            nc.vector.scalar_tensor_tensor(out=avn[0:ksh, :], in0=av[0:ksh, :],
                                           scalar=4.0, in1=rsb[0:ksh, 0:256],
                                           op0=Alu.mult, op1=Alu.add)
            nc.vector.tensor_scalar_mul(avn[0:ksh, :], avn[0:ksh, :], 0.2)
            den = opool.tile([128, 1], F32)
            nc.vector.tensor_scalar_add(den[0:ksh, :], csn[0:ksh, :], EPS)
            rec = opool.tile([128, 1], F32)
            nc.vector.reciprocal(rec[0:ksh, :], den[0:ksh, :])
            nc.vector.tensor_scalar(out=emn[0:ksh, :], in0=avn[0:ksh, :],
                                    scalar1=rec[0:ksh, :], scalar2=None,
                                    op0=Alu.mult)
            nc.sync.dma_start(csn_d[:].rearrange("(a b) -> a b", b=1),
                              csn[0:ksh, 0:1])
            nc.sync.dma_start(avn_d[:], avn[0:ksh, :])
            nc.sync.dma_start(emn_d[:], emn[0:ksh, :])

    nc.compile()
    return nc


_NC_CACHE = {}


def _get_nc():
    key = (N_CORES, NTILES)
    if key not in _NC_CACHE:
        _NC_CACHE[key] = build_kernel(*key)
    return _NC_CACHE[key]


LAST_RESULTS = None


def kernel(x, embed, cluster_size, embed_avg, _trace=False):
    global LAST_RESULTS
    nc = _get_nc()
    xf = np.ascontiguousarray(np.asarray(x).reshape(-1, D), dtype=np.float32)
    emb = np.ascontiguousarray(embed, np.float32)
    in_maps = []
    for c in range(N_CORES):
        in_maps.append({"x_sh": xf[c * NT:(c + 1) * NT], "embed": emb})
    res = run_bass_kernel_spmd(nc, in_maps, core_ids=list(range(N_CORES)),
                               trace=_trace)
    LAST_RESULTS = res
    outs = res.results
    quant = np.concatenate([r["quant_sh"] for r in outs]).reshape(B, T, D)
    ind = np.concatenate(
        [r["ind_sh"].T.reshape(-1) for r in outs]).reshape(B, T)
    # cross-core reduction of per-core [K, 257] segment-sum partials + EMA
    seg = np.sum([r["part_sh"] for r in outs], axis=0, dtype=np.float32)
    counts = seg[:, 256]
    esum = seg[:, 0:256]
    cs_new = (np.float32(DECAY) * np.asarray(cluster_size, np.float32)
              + np.float32(1.0 - DECAY) * counts)
    av_new = (np.float32(DECAY) * np.asarray(embed_avg, np.float32)
              + np.float32(1.0 - DECAY) * esum)
    em_new = av_new / (cs_new + np.float32(EPS))[:, None]
    return quant, ind.astype(np.int32), cs_new, av_new, em_new


# revision 28
# speedup vs baseline: 1.2179x; 1.2179x over previous
"""EuclideanCodebook (VQ) Trainium2 Bass kernel.

Strategy (data-parallel over 8 NeuronCores, x sharded on tokens):
  per 128-token tile:
    - scores s[t,k] = 2*x.e_k  via fp16 hi/lo split matmuls (3 passes, exact to
      ~2^-22) accumulated in PSUM fp32;
    - fused DVE tensor_tensor_reduce: adj = s - |e_k|^2, m[t] = max_k adj
    - onehot = (adj >= m) on DVE (exact fp32 compare), fp16
    - ind[t] = sum_k onehot*iota  (scalar_tensor_tensor with accum)
    - quantize rows gathered from embed DRAM via indirect DMA
    - segment sums: onehot.T @ [x_h] accumulated in PSUM across all tiles
      (8 k-chunks) + count column via onehot.T @ ones
  epilogue: ReduceScatter(sum) of [1024,257] partials over 8 cores; each core
  EMA-updates its 128-row shard of cluster_size/embed_avg/embed.
Host side only shards/concats and reorders embed_ind.
"""
import numpy as np
from contextlib import ExitStack

import concourse.bass as bass
import concourse.bacc as bacc
import concourse.mybir as mybir
import concourse.tile as tile
from concourse.bass_utils import run_bass_kernel_spmd

dt = mybir.dt
F32 = dt.float32
F16 = dt.float16
I32 = dt.int32
Alu = mybir.AluOpType
Act = mybir.ActivationFunctionType

# problem shapes (hardcoded per contract)
B, T, D, K = 8, 8192, 256, 1024
N_CORES = 8
NT = B * T // N_CORES          # tokens per core (8192)
NTILES = NT // 128             # 64
KC = K // 128                  # 8 k-chunks
KSH = K // N_CORES             # 128 codes per core for EMA shard
DECAY, EPS = 0.8, 1e-5


def build_kernel(n_cores=N_CORES, ntiles=NTILES, use_cc=False):
    nt = ntiles * 128
    ksh = K // n_cores
    nc = bacc.Bacc("TRN2", target_bir_lowering=False, debug=False,
                   num_devices=n_cores)

    x_d = nc.dram_tensor("x_sh", [nt, D], F32, kind="ExternalInput")
    e_d = nc.dram_tensor("embed", [K, D], F32, kind="ExternalInput")
    if use_cc:
        cs_d = nc.dram_tensor("cs_sh", [ksh], F32, kind="ExternalInput")
        av_d = nc.dram_tensor("av_sh", [ksh, D], F32, kind="ExternalInput")

    q_d = nc.dram_tensor("quant_sh", [nt, D], F32, kind="ExternalOutput")
    ind_d = nc.dram_tensor("ind_sh", [128, ntiles], I32, kind="ExternalOutput")
    if use_cc:
        csn_d = nc.dram_tensor("cs_new_sh", [ksh], F32, kind="ExternalOutput")
        avn_d = nc.dram_tensor("av_new_sh", [ksh, D], F32,
                               kind="ExternalOutput")
        emn_d = nc.dram_tensor("em_new_sh", [ksh, D], F32,
                               kind="ExternalOutput")
    else:
        part_d = nc.dram_tensor("part_sh", [K, 257], F32,
                                kind="ExternalOutput")

    with tile.TileContext(nc) as tc, ExitStack() as ctx:
        cpool = ctx.enter_context(tc.tile_pool(name="consts", bufs=1))
        spool = ctx.enter_context(tc.tile_pool(name="setup", bufs=1))
        xpool = ctx.enter_context(tc.tile_pool(name="x", bufs=4))
        wpool = ctx.enter_context(tc.tile_pool(name="work", bufs=3))
        qpool = ctx.enter_context(tc.tile_pool(name="q", bufs=4))
        opool = ctx.enter_context(tc.tile_pool(name="out1", bufs=1))
        ps_s = ctx.enter_context(tc.tile_pool(name="pscore", bufs=3,
                                              space="PSUM"))
        ps_a = ctx.enter_context(tc.tile_pool(name="pacc", bufs=1,
                                              space="PSUM"))
        dpool = ctx.enter_context(tc.tile_pool(name="dram", bufs=1,
                                               space="DRAM"))

        # ---------------- constants / setup ----------------
        embT_h0 = cpool.tile([128, K], F16)   # d 0:128, k, fp16 hi of 2*e^T
        embT_h1 = cpool.tile([128, K], F16)   # d 128:256
        embT_l0 = cpool.tile([128, K], F16)
        embT_l1 = cpool.tile([128, K], F16)
        e2row = cpool.tile([1, K], F32)
        e2full = cpool.tile([128, K], F32)
        iota_row = cpool.tile([1, K], F16)
        iotafull = cpool.tile([128, K], F16)
        ones_r32 = cpool.tile([1, 128], F32)
        ones_r16 = cpool.tile([1, 128], F16)
        ident32 = cpool.tile([128, 128], F32)
        ident16 = cpool.tile([128, 128], F16)
        ones_c32 = cpool.tile([128, 1], F32)
        ones_c16 = cpool.tile([128, 1], F16)
        indacc = opool.tile([128, ntiles], I32)

        # persistent PSUM: scatter accumulator (4 banks) + counts (1 bank)
        pscat = ps_a.tile([128, 2048], F32)
        pcnt = ps_a.tile([128, 512], F32)

        ones128 = spool.tile([128, 128], F32)
        nc.vector.memset(ones128[:], 1.0)
        nc.gpsimd.affine_select(ident32[:], ones128[:], pattern=[[-1, 128]],
                                base=0, channel_multiplier=1,
                                compare_op=Alu.is_equal, fill=0.0)
        nc.vector.tensor_copy(ident16[:], ident32[:])
        nc.vector.memset(ones_c32[:], 1.0)
        nc.vector.memset(ones_c16[:], 1.0)

        iota_i = spool.tile([1, K], I32)
        nc.gpsimd.iota(iota_i[:], pattern=[[1, K]], base=0,
                       channel_multiplier=0)
        nc.vector.tensor_copy(iota_row[:], iota_i[:])
        nc.vector.memset(ones_r32[:], 1.0)
        nc.vector.memset(ones_r16[:], 1.0)
        # broadcast iota to all partitions: ones[1,128].T @ iota_row
        for h in range(2):
            ks = slice(h * 512, (h + 1) * 512)
            pb = ps_s.tile([128, 512], F32, name="pb", tag="ps")
            nc.tensor.matmul(pb[:], ones_r16[:], iota_row[0:1, ks])
            nc.vector.tensor_copy(iotafull[:, ks], pb[:])

        # embed transpose + split + e2
        et = spool.tile([128, D], F32)
        t2 = spool.tile([128, D], F32)
        sq = spool.tile([128, 128], F32)
        for c in range(KC):
            et_ = spool.tile([128, D], F32, name=f"et{c}", tag="et")
            nc.sync.dma_start(et_[:], e_d[c * 128:(c + 1) * 128, :])
            # transpose both d-halves into pcnt[:, 0:256] (exact fp32 e^T);
            # single psum group per chunk (2nd start would re-zero the bank)
            nc.tensor.matmul(pcnt[:, 0:128], et_[:, 0:128], ident32[:],
                             is_transpose=True, start=True, stop=False)
            nc.tensor.matmul(pcnt[:, 128:256], et_[:, 128:256], ident32[:],
                             is_transpose=True, start=False, stop=True)
            t2_ = spool.tile([128, D], F32, name=f"t2{c}", tag="t2")
            nc.scalar.mul(t2_[:], pcnt[:, 0:256], 2.0)
            kk = slice(c * 128, (c + 1) * 128)
            nc.scalar.copy(embT_h0[:, kk], t2_[:, 0:128])
            nc.scalar.copy(embT_h1[:, kk], t2_[:, 128:256])
            nc.vector.tensor_tensor(out=embT_l0[:, kk], in0=t2_[:, 0:128],
                                    in1=embT_h0[:, kk], op=Alu.subtract)
            nc.vector.tensor_tensor(out=embT_l1[:, kk], in0=t2_[:, 128:256],
                                    in1=embT_h1[:, kk], op=Alu.subtract)
            # e2 contribution: sum_d e^2 = ones.T @ (eT*eT) ; eT in pcnt.
            # one psum group per 512-wide bank (chunks 4c..4c+3)
            sq0 = spool.tile([128, 128], F32, name=f"sq0{c}", tag="sq0")
            sq1 = spool.tile([128, 128], F32, name=f"sq1{c}", tag="sq1")
            nc.scalar.square(sq0[:], pcnt[:, 0:128])
            nc.scalar.square(sq1[:], pcnt[:, 128:256])
            nc.tensor.matmul(pscat[0:1, kk], ones_c32[:], sq0[:],
                             start=(c % 4 == 0), stop=False)
            nc.tensor.matmul(pscat[0:1, kk], ones_c32[:], sq1[:],
                             start=False, stop=(c % 4 == 3))
        nc.vector.tensor_copy(e2row[0:1, :], pscat[0:1, 0:K])
        # broadcast e2 to all partitions (fp32 matmul, 512-wide chunks)
        for h in range(2):
            ks = slice(h * 512, (h + 1) * 512)
            pb2 = ps_s.tile([128, 512], F32, name="pb2", tag="ps")
            nc.tensor.matmul(pb2[:], ones_r32[:], e2row[0:1, ks])
            nc.vector.tensor_copy(e2full[:, ks], pb2[:])

        # ---------------- main loop ----------------
        for t in range(ntiles):
            rows = slice(t * 128, (t + 1) * 128)
            xt = xpool.tile([128, D], F32, name="xt", tag="xt")
            nc.sync.dma_start(xt[:], x_d[rows, :])
            # fp16 split
            xaug = xpool.tile([128, 258], F16, name="xaug", tag="xaug")
            nc.scalar.copy(xaug[:, 0:256], xt[:])
            nc.vector.memset(xaug[:, 256:258], 1.0)
            xl = xpool.tile([128, D], F16, name="xl", tag="xl")
            nc.vector.tensor_tensor(out=xl[:], in0=xt[:], in1=xaug[:, 0:256],
                                    op=Alu.subtract)
            # transposes (fp16) into a rotating score-pool bank viewed as f16:
            # [0:256]f16 = xh^T, [256:512]f16 = xl^T. One psum group of 4.
            pxt = ps_s.tile([128, 512], F32, name="pxt", tag="ps")
            pf16 = pxt[:].bitcast(F16)
            nc.tensor.matmul(pf16[:, 0:128], xaug[:, 0:128], ident16[:],
                             is_transpose=True, start=True, stop=False)
            nc.tensor.matmul(pf16[:, 128:256], xaug[:, 128:256], ident16[:],
                             is_transpose=True, start=False, stop=False)
            nc.tensor.matmul(pf16[:, 256:384], xl[:, 0:128], ident16[:],
                             is_transpose=True, start=False, stop=False)
            nc.tensor.matmul(pf16[:, 384:512], xl[:, 128:256], ident16[:],
                             is_transpose=True, start=False, stop=True)
            xth = xpool.tile([128, D], F16, name="xth", tag="xth")
            xtl = xpool.tile([128, D], F16, name="xtl", tag="xtl")
            nc.scalar.copy(xth[:], pf16[:, 0:256])
            nc.scalar.copy(xtl[:], pf16[:, 256:512])

            adj = wpool.tile([128, K], F32, name="adj", tag="adj")
            m01 = wpool.tile([128, 2], F32, name="m01", tag="m01")
            # 12 accumulating matmuls ordered for stationary-weight reuse:
            # each weight (xth/xtl d-chunk) serves its rhs over both k-halves
            plan = [(xth, 0, embT_h0), (xth, 1, embT_h1),
                    (xth, 0, embT_l0), (xth, 1, embT_l1),
                    (xtl, 0, embT_h0), (xtl, 1, embT_h1)]
            for h in range(2):
                ks = slice(h * 512, (h + 1) * 512)
                ps = ps_s.tile([128, 512], F32, name="ps", tag="ps")
                for i, (w, dc, rhs) in enumerate(plan):
                    nc.tensor.matmul(ps[:], w[:, dc * 128:(dc + 1) * 128],
                                     rhs[:, ks], start=(i == 0),
                                     stop=(i == len(plan) - 1))
                nc.vector.tensor_tensor(out=adj[:, ks], in0=ps[:],
                                        in1=e2full[:, ks], op=Alu.subtract)
                nc.vector.tensor_reduce(out=m01[:, h:h + 1], in_=adj[:, ks],
                                        axis=mybir.AxisListType.X, op=Alu.max)
            m = wpool.tile([128, 1], F32, name="m", tag="m")
            nc.vector.tensor_tensor(out=m[:], in0=m01[:, 0:1],
                                    in1=m01[:, 1:2], op=Alu.max)
            # complement onehot on ACT: Sign(m - adj) = {0 hit, +1 miss};
            # scatter then yields colsum - seg, fixed up on the host.
            oh = wpool.tile([128, K], F16, name="oh", tag="oh")
            nc.scalar.activation(oh[:], adj[:], Act.Sign, bias=m[:],
                                 scale=-1.0)
            # index extraction: hits are 0 -> (oh <= 0.5) * iota
            indf = wpool.tile([128, 1], F32, name="indf", tag="indf")
            ohs = wpool.tile([128, K], F16, name="ohs", tag="ohs")
            nc.vector.scalar_tensor_tensor(
                out=ohs[:], in0=oh[:], scalar=0.5,
                in1=iotafull[:],
                op0=Alu.is_le, op1=Alu.mult, accum_out=indf[:])
            indi = wpool.tile([128, 1], I32, name="indi", tag="indi")
            nc.vector.tensor_copy(indi[:], indf[:])
            nc.vector.tensor_copy(indacc[:, t:t + 1], indi[:])
            # quantize gather: embed rows by index (DRAM -> SBUF -> DRAM)
            q = qpool.tile([128, D], F32, name="q", tag="q")
            nc.gpsimd.indirect_dma_start(
                out=q[:], out_offset=None, in_=e_d[:],
                in_offset=bass.IndirectOffsetOnAxis(ap=indi[:], axis=0))
            nc.sync.dma_start(q_d[rows, :], q[:])
            # scatter: segment sums (8 k-chunks, 2 chunks share a psum bank
            # -> one group per bank) + counts (all 8 cols in one bank/group)
            for c in range(KC):
                kk = slice(c * 128, (c + 1) * 128)
                nc.tensor.matmul(pscat[:, c * 256:(c + 1) * 256],
                                 oh[:, kk], xaug[:, 0:256],
                                 start=(t == 0 and c % 2 == 0),
                                 stop=(t == ntiles - 1 and c % 2 == 1))
                nc.tensor.matmul(pcnt[:, c:c + 1],
                                 oh[:, kk], xaug[:, 256:257],
                                 start=(t == 0 and c == 0),
                                 stop=(t == ntiles - 1 and c == KC - 1))

        nc.sync.dma_start(ind_d[:], indacc[:])

        # ---------------- epilogue: reduce + EMA ----------------
        if not use_cc:
            # flush per-core partial sums; host reduces across cores
            for c in range(KC):
                st = wpool.tile([128, 257], F32, name="st", tag="st")
                nc.scalar.copy(st[:, 0:256], pscat[:, c * 256:(c + 1) * 256])
                nc.vector.tensor_copy(st[:, 256:257], pcnt[:, c:c + 1])
                nc.sync.dma_start(part_d[c * 128:(c + 1) * 128, :], st[:])
        else:
            rs_in = dpool.tile([K, 257], F32)
            rs_out = dpool.tile([K // n_cores, 257], F32)
            for c in range(KC):
                st = wpool.tile([128, 257], F32, name="st", tag="st")
                nc.scalar.copy(st[:, 0:256], pscat[:, c * 256:(c + 1) * 256])
                nc.vector.tensor_copy(st[:, 256:257], pcnt[:, c:c + 1])
                nc.sync.dma_start(rs_in[c * 128:(c + 1) * 128, :], st[:])
            nc.gpsimd.collective_compute(
                "ReduceScatter", Alu.add,
                replica_groups=[list(range(n_cores))],
                ins=[rs_in.opt()], outs=[rs_out.opt()])

            rsb = opool.tile([128, 257], F32)
            nc.sync.dma_start(rsb[0:ksh, :], rs_out[:])
            cs = opool.tile([128, 1], F32)
            av = opool.tile([128, D], F32)
            nc.sync.dma_start(cs[0:ksh, 0:1],
                              cs_d[:].rearrange("(a b) -> a b", b=1))
            nc.sync.dma_start(av[0:ksh, :], av_d[:])
            csn = opool.tile([128, 1], F32)
            avn = opool.tile([128, D], F32)
            emn = opool.tile([128, D], F32)
            # new = (old*4 + seg) * 0.2  == old*0.8 + seg*0.2
            nc.vector.scalar_tensor_tensor(out=csn[0:ksh, :], in0=cs[0:ksh, :],
                                           scalar=4.0, in1=rsb[0:ksh, 256:257],
                                           op0=Alu.mult, op1=Alu.add)
            nc.vector.tensor_scalar_mul(csn[0:ksh, :], csn[0:ksh, :], 0.2)
            nc.vector.scalar_tensor_tensor(out=avn[0:ksh, :], in0=av[0:ksh, :],
                                           scalar=4.0, in1=rsb[0:ksh, 0:256],
                                           op0=Alu.mult, op1=Alu.add)
            nc.vector.tensor_scalar_mul(avn[0:ksh, :], avn[0:ksh, :], 0.2)
            den = opool.tile([128, 1], F32)
            nc.vector.tensor_scalar_add(den[0:ksh, :], csn[0:ksh, :], EPS)
            rec = opool.tile([128, 1], F32)
            nc.vector.reciprocal(rec[0:ksh, :], den[0:ksh, :])
            nc.vector.tensor_scalar(out=emn[0:ksh, :], in0=avn[0:ksh, :],
                                    scalar1=rec[0:ksh, :], scalar2=None,
                                    op0=Alu.mult)
            nc.sync.dma_start(csn_d[:].rearrange("(a b) -> a b", b=1),
                              csn[0:ksh, 0:1])
            nc.sync.dma_start(avn_d[:], avn[0:ksh, :])
            nc.sync.dma_start(emn_d[:], emn[0:ksh, :])

    nc.compile()
    return nc


_NC_CACHE = {}


def _get_nc():
    key = (N_CORES, NTILES)
    if key not in _NC_CACHE:
        _NC_CACHE[key] = build_kernel(*key)
    return _NC_CACHE[key]


LAST_RESULTS = None


def kernel(x, embed, cluster_size, embed_avg, _trace=False):
    global LAST_RESULTS
    nc = _get_nc()
    xf = np.ascontiguousarray(np.asarray(x).reshape(-1, D), dtype=np.float32)
    emb = np.ascontiguousarray(embed, np.float32)
    in_maps = []
    for c in range(N_CORES):
        in_maps.append({"x_sh": xf[c * NT:(c + 1) * NT], "embed": emb})
    res = run_bass_kernel_spmd(nc, in_maps, core_ids=list(range(N_CORES)),
                               trace=_trace)
    LAST_RESULTS = res
    outs = res.results
    quant = np.concatenate([r["quant_sh"] for r in outs]).reshape(B, T, D)
    ind = np.concatenate(
        [r["ind_sh"].T.reshape(-1) for r in outs]).reshape(B, T)
    # cross-core reduction of per-core [K, 257] complement partials + EMA:
    # device computed sum_t (1-onehot)*[x_h|1], so seg = colsum - partials
    comp = np.sum([r["part_sh"] for r in outs], axis=0, dtype=np.float64)
    xh = xf.astype(np.float16).astype(np.float64)
    colsum = np.concatenate([xh.sum(0), [float(xf.shape[0])]])
    seg = (colsum[None, :] - comp).astype(np.float32)
    counts = seg[:, 256]
    esum = seg[:, 0:256]
    cs_new = (np.float32(DECAY) * np.asarray(cluster_size, np.float32)
              + np.float32(1.0 - DECAY) * counts)
    av_new = (np.float32(DECAY) * np.asarray(embed_avg, np.float32)
              + np.float32(1.0 - DECAY) * esum)
    em_new = av_new / (cs_new + np.float32(EPS))[:, None]
    return quant, ind.astype(np.int32), cs_new, av_new, em_new


# revision 40
# speedup vs baseline: 1.2418x; 1.0196x over previous
"""EuclideanCodebook (VQ) Trainium2 Bass kernel.

Strategy (data-parallel over 8 NeuronCores, x sharded on tokens):
  per 128-token tile:
    - scores s[t,k] = 2*x.e_k  via fp16 hi/lo split matmuls (3 passes, exact to
      ~2^-22) accumulated in PSUM fp32;
    - fused DVE tensor_tensor_reduce: adj = s - |e_k|^2, m[t] = max_k adj
    - onehot = (adj >= m) on DVE (exact fp32 compare), fp16
    - ind[t] = sum_k onehot*iota  (scalar_tensor_tensor with accum)
    - quantize rows gathered from embed DRAM via indirect DMA
    - segment sums: onehot.T @ [x_h] accumulated in PSUM across all tiles
      (8 k-chunks) + count column via onehot.T @ ones
  epilogue: ReduceScatter(sum) of [1024,257] partials over 8 cores; each core
  EMA-updates its 128-row shard of cluster_size/embed_avg/embed.
Host side only shards/concats and reorders embed_ind.
"""
import numpy as np
from contextlib import ExitStack

import concourse.bass as bass
import concourse.bacc as bacc
import concourse.mybir as mybir
import concourse.tile as tile
from concourse.bass_utils import run_bass_kernel_spmd

dt = mybir.dt
F32 = dt.float32
F16 = dt.float16
I32 = dt.int32
Alu = mybir.AluOpType
Act = mybir.ActivationFunctionType

# problem shapes (hardcoded per contract)
B, T, D, K = 8, 8192, 256, 1024
N_CORES = 8
NT = B * T // N_CORES          # tokens per core (8192)
NTILES = NT // 128             # 64
KC = K // 128                  # 8 k-chunks
KSH = K // N_CORES             # 128 codes per core for EMA shard
DECAY, EPS = 0.8, 1e-5


def build_kernel(n_cores=N_CORES, ntiles=NTILES, use_cc=False):
    nt = ntiles * 128
    ksh = K // n_cores
    nc = bacc.Bacc("TRN2", target_bir_lowering=False, debug=False,
                   num_devices=n_cores)

    x_d = nc.dram_tensor("x_sh", [nt, D], F32, kind="ExternalInput")
    e_d = nc.dram_tensor("embed", [K, D], F32, kind="ExternalInput")
    if use_cc:
        cs_d = nc.dram_tensor("cs_sh", [ksh], F32, kind="ExternalInput")
        av_d = nc.dram_tensor("av_sh", [ksh, D], F32, kind="ExternalInput")

    q_d = nc.dram_tensor("quant_sh", [nt, D], F32, kind="ExternalOutput")
    ind_d = nc.dram_tensor("ind_sh", [128, ntiles], I32, kind="ExternalOutput")
    if use_cc:
        csn_d = nc.dram_tensor("cs_new_sh", [ksh], F32, kind="ExternalOutput")
        avn_d = nc.dram_tensor("av_new_sh", [ksh, D], F32,
                               kind="ExternalOutput")
        emn_d = nc.dram_tensor("em_new_sh", [ksh, D], F32,
                               kind="ExternalOutput")
    else:
        part_d = nc.dram_tensor("part_sh", [K, 256], F32,
                                kind="ExternalOutput")

    with tile.TileContext(nc) as tc, ExitStack() as ctx:
        cpool = ctx.enter_context(tc.tile_pool(name="consts", bufs=1))
        spool = ctx.enter_context(tc.tile_pool(name="setup", bufs=1))
        xpool = ctx.enter_context(tc.tile_pool(name="x", bufs=4))
        wpool = ctx.enter_context(tc.tile_pool(name="work", bufs=3))
        qpool = ctx.enter_context(tc.tile_pool(name="q", bufs=4))
        opool = ctx.enter_context(tc.tile_pool(name="out1", bufs=1))
        ps_s = ctx.enter_context(tc.tile_pool(name="pscore", bufs=4,
                                              space="PSUM"))
        ps_a = ctx.enter_context(tc.tile_pool(name="pacc", bufs=1,
                                              space="PSUM"))
        dpool = ctx.enter_context(tc.tile_pool(name="dram", bufs=1,
                                               space="DRAM"))

        # ---------------- constants / setup ----------------
        embT_h0 = cpool.tile([128, K], F16)   # d 0:128, k, fp16 hi of 2*e^T
        embT_h1 = cpool.tile([128, K], F16)   # d 128:256
        embT_l0 = cpool.tile([128, K], F16)
        embT_l1 = cpool.tile([128, K], F16)
        e2row = cpool.tile([1, K], F32)
        e2full = cpool.tile([128, K], F32)
        iota_row = cpool.tile([1, K], F16)
        iotafull = cpool.tile([128, K], F16)
        ones_r32 = cpool.tile([1, 128], F32)
        ones_r16 = cpool.tile([1, 128], F16)
        ident32 = cpool.tile([128, 128], F32)
        ident16 = cpool.tile([128, 128], F16)
        ones_c32 = cpool.tile([128, 1], F32)
        ones_c16 = cpool.tile([128, 1], F16)
        indacc = opool.tile([128, ntiles], I32)

        # persistent PSUM: scatter accumulator (4 banks); counts are derived
        # host-side from the embed_ind output (bincount)
        pscat = ps_a.tile([128, 2048], F32)

        ones128 = spool.tile([128, 128], F32)
        nc.vector.memset(ones128[:], 1.0)
        nc.gpsimd.affine_select(ident32[:], ones128[:], pattern=[[-1, 128]],
                                base=0, channel_multiplier=1,
                                compare_op=Alu.is_equal, fill=0.0)
        nc.vector.tensor_copy(ident16[:], ident32[:])
        nc.vector.memset(ones_c32[:], 1.0)
        nc.vector.memset(ones_c16[:], 1.0)

        iota_i = spool.tile([1, K], I32)
        nc.gpsimd.iota(iota_i[:], pattern=[[1, K]], base=0,
                       channel_multiplier=0)
        nc.vector.tensor_copy(iota_row[:], iota_i[:])
        nc.vector.memset(ones_r32[:], 1.0)
        nc.vector.memset(ones_r16[:], 1.0)
        # broadcast iota to all partitions: ones[1,128].T @ iota_row
        for h in range(2):
            ks = slice(h * 512, (h + 1) * 512)
            pb = ps_s.tile([128, 512], F32, name="pb", tag="ps")
            nc.tensor.matmul(pb[:], ones_r16[:], iota_row[0:1, ks])
            nc.vector.tensor_copy(iotafull[:, ks], pb[:])

        # embed transpose + split + e2
        et = spool.tile([128, D], F32)
        t2 = spool.tile([128, D], F32)
        sq = spool.tile([128, 128], F32)
        for c in range(KC):
            et_ = spool.tile([128, D], F32, name=f"et{c}", tag="et")
            nc.sync.dma_start(et_[:], e_d[c * 128:(c + 1) * 128, :])
            # transpose both d-halves into a score-pool bank (exact fp32 e^T);
            # single psum group per chunk (2nd start would re-zero the bank)
            pet = ps_s.tile([128, 512], F32, name="pet", tag="ps")
            nc.tensor.matmul(pet[:, 0:128], et_[:, 0:128], ident32[:],
                             is_transpose=True, start=True, stop=False)
            nc.tensor.matmul(pet[:, 128:256], et_[:, 128:256], ident32[:],
                             is_transpose=True, start=False, stop=True)
            t2_ = spool.tile([128, D], F32, name=f"t2{c}", tag="t2")
            nc.scalar.mul(t2_[:], pet[:, 0:256], 2.0)
            kk = slice(c * 128, (c + 1) * 128)
            nc.scalar.copy(embT_h0[:, kk], t2_[:, 0:128])
            nc.scalar.copy(embT_h1[:, kk], t2_[:, 128:256])
            nc.vector.tensor_tensor(out=embT_l0[:, kk], in0=t2_[:, 0:128],
                                    in1=embT_h0[:, kk], op=Alu.subtract)
            nc.vector.tensor_tensor(out=embT_l1[:, kk], in0=t2_[:, 128:256],
                                    in1=embT_h1[:, kk], op=Alu.subtract)
            # e2 contribution: sum_d e^2 = ones.T @ (eT*eT)
            # one psum group per 512-wide bank (chunks 4c..4c+3)
            sq0 = spool.tile([128, 128], F32, name=f"sq0{c}", tag="sq0")
            sq1 = spool.tile([128, 128], F32, name=f"sq1{c}", tag="sq1")
            nc.scalar.square(sq0[:], pet[:, 0:128])
            nc.scalar.square(sq1[:], pet[:, 128:256])
            nc.tensor.matmul(pscat[0:1, kk], ones_c32[:], sq0[:],
                             start=(c % 4 == 0), stop=False)
            nc.tensor.matmul(pscat[0:1, kk], ones_c32[:], sq1[:],
                             start=False, stop=(c % 4 == 3))
        nc.vector.tensor_copy(e2row[0:1, :], pscat[0:1, 0:K])
        # broadcast e2 to all partitions (fp32 matmul, 512-wide chunks)
        for h in range(2):
            ks = slice(h * 512, (h + 1) * 512)
            pb2 = ps_s.tile([128, 512], F32, name="pb2", tag="ps")
            nc.tensor.matmul(pb2[:], ones_r32[:], e2row[0:1, ks])
            nc.vector.tensor_copy(e2full[:, ks], pb2[:])

        # ---------------- main loop ----------------
        # the scatter matmuls for tile t are emitted during iteration t+1 so
        # the PE never stalls waiting for tile t's onehot (ACT) to land
        def emit_scatter(t, oh, xaug):
            for c in range(KC):
                kk = slice(c * 128, (c + 1) * 128)
                nc.tensor.matmul(pscat[:, c * 256:(c + 1) * 256],
                                 oh[:, kk], xaug[:, 0:256],
                                 start=(t == 0 and c % 2 == 0),
                                 stop=(t == ntiles - 1 and c % 2 == 1))

        prev = None
        for t in range(ntiles):
            rows = slice(t * 128, (t + 1) * 128)
            xt = xpool.tile([128, D], F32, name="xt", tag="xt")
            nc.sync.dma_start(xt[:], x_d[rows, :])
            # fp16 split
            xaug = xpool.tile([128, 256], F16, name="xaug", tag="xaug")
            nc.scalar.copy(xaug[:, 0:256], xt[:])
            xl = xpool.tile([128, D], F16, name="xl", tag="xl")
            nc.vector.tensor_tensor(out=xl[:], in0=xt[:], in1=xaug[:, 0:256],
                                    op=Alu.subtract)
            # transposes (fp16) into a rotating score-pool bank viewed as f16:
            # [0:256]f16 = xh^T, [256:512]f16 = xl^T. One psum group of 4.
            pxt = ps_s.tile([128, 512], F32, name="pxt", tag="ps")
            pf16 = pxt[:].bitcast(F16)
            nc.tensor.matmul(pf16[:, 0:128], xaug[:, 0:128], ident16[:],
                             is_transpose=True, start=True, stop=False)
            nc.tensor.matmul(pf16[:, 128:256], xaug[:, 128:256], ident16[:],
                             is_transpose=True, start=False, stop=False)
            nc.tensor.matmul(pf16[:, 256:384], xl[:, 0:128], ident16[:],
                             is_transpose=True, start=False, stop=False)
            nc.tensor.matmul(pf16[:, 384:512], xl[:, 128:256], ident16[:],
                             is_transpose=True, start=False, stop=True)
            xth = xpool.tile([128, D], F16, name="xth", tag="xth")
            xtl = xpool.tile([128, D], F16, name="xtl", tag="xtl")
            nc.scalar.copy(xth[:], pf16[:, 0:256])
            nc.scalar.copy(xtl[:], pf16[:, 256:512])

            adj = wpool.tile([128, K], F32, name="adj", tag="adj")
            m01 = wpool.tile([128, 2], F32, name="m01", tag="m01")
            # 12 accumulating matmuls ordered for stationary-weight reuse:
            # each weight (xth/xtl d-chunk) serves its rhs over both k-halves
            plan = [(xth, 0, embT_h0), (xth, 1, embT_h1),
                    (xth, 0, embT_l0), (xth, 1, embT_l1),
                    (xtl, 0, embT_h0), (xtl, 1, embT_h1)]
            for h in range(2):
                ks = slice(h * 512, (h + 1) * 512)
                ps = ps_s.tile([128, 512], F32, name="ps", tag="ps")
                for i, (w, dc, rhs) in enumerate(plan):
                    nc.tensor.matmul(ps[:], w[:, dc * 128:(dc + 1) * 128],
                                     rhs[:, ks], start=(i == 0),
                                     stop=(i == len(plan) - 1))
                nc.vector.tensor_tensor(out=adj[:, ks], in0=ps[:],
                                        in1=e2full[:, ks], op=Alu.subtract)
                nc.vector.tensor_reduce(out=m01[:, h:h + 1], in_=adj[:, ks],
                                        axis=mybir.AxisListType.X, op=Alu.max)
            if prev is not None:
                emit_scatter(t - 1, *prev)
            m = wpool.tile([128, 1], F32, name="m", tag="m")
            nc.vector.tensor_tensor(out=m[:], in0=m01[:, 0:1],
                                    in1=m01[:, 1:2], op=Alu.max)
            # complement onehot on ACT: Sign(m - adj) = {0 hit, +1 miss};
            # scatter then yields colsum - seg, fixed up on the host.
            oh = wpool.tile([128, K], F16, name="oh", tag="oh")
            nc.scalar.activation(oh[:], adj[:], Act.Sign, bias=m[:],
                                 scale=-1.0)
            # index extraction: hits are 0 -> (oh <= 0.5) * iota
            indf = wpool.tile([128, 1], F32, name="indf", tag="indf")
            ohs = wpool.tile([128, K], F16, name="ohs", tag="ohs")
            nc.vector.scalar_tensor_tensor(
                out=ohs[:], in0=oh[:], scalar=0.5,
                in1=iotafull[:],
                op0=Alu.is_le, op1=Alu.mult, accum_out=indf[:])
            indi = wpool.tile([128, 1], I32, name="indi", tag="indi")
            nc.vector.tensor_copy(indi[:], indf[:])
            nc.vector.tensor_copy(indacc[:, t:t + 1], indi[:])
            # quantize gather: embed rows by index (DRAM -> SBUF -> DRAM)
            q = qpool.tile([128, D], F32, name="q", tag="q")
            nc.gpsimd.indirect_dma_start(
                out=q[:], out_offset=None, in_=e_d[:],
                in_offset=bass.IndirectOffsetOnAxis(ap=indi[:], axis=0))
            nc.sync.dma_start(q_d[rows, :], q[:])
            prev = (oh, xaug)

        emit_scatter(ntiles - 1, *prev)
        nc.sync.dma_start(ind_d[:], indacc[:])

        # ---------------- epilogue: reduce + EMA ----------------
        if not use_cc:
            # flush per-core partial sums; host reduces across cores
            for c in range(KC):
                st = wpool.tile([128, 256], F32, name="st", tag="st")
                nc.scalar.copy(st[:], pscat[:, c * 256:(c + 1) * 256])
                nc.sync.dma_start(part_d[c * 128:(c + 1) * 128, :], st[:])
        else:
            # NOTE: collective_compute hangs under the axon/PJRT path in this
            # container; this branch is kept for native-NRT environments.
            rs_in = dpool.tile([K, 257], F32)
            rs_out = dpool.tile([K // n_cores, 257], F32)
            for c in range(KC):
                st = wpool.tile([128, 257], F32, name="st", tag="st")
                nc.scalar.copy(st[:, 0:256], pscat[:, c * 256:(c + 1) * 256])
                nc.vector.memset(st[:, 256:257], 0.0)
                nc.sync.dma_start(rs_in[c * 128:(c + 1) * 128, :], st[:])
            nc.gpsimd.collective_compute(
                "ReduceScatter", Alu.add,
                replica_groups=[list(range(n_cores))],
                ins=[rs_in.opt()], outs=[rs_out.opt()])

            rsb = opool.tile([128, 257], F32)
            nc.sync.dma_start(rsb[0:ksh, :], rs_out[:])
            cs = opool.tile([128, 1], F32)
            av = opool.tile([128, D], F32)
            nc.sync.dma_start(cs[0:ksh, 0:1],
                              cs_d[:].rearrange("(a b) -> a b", b=1))
            nc.sync.dma_start(av[0:ksh, :], av_d[:])
            csn = opool.tile([128, 1], F32)
            avn = opool.tile([128, D], F32)
            emn = opool.tile([128, D], F32)
            # new = (old*4 + seg) * 0.2  == old*0.8 + seg*0.2
            nc.vector.scalar_tensor_tensor(out=csn[0:ksh, :], in0=cs[0:ksh, :],
                                           scalar=4.0, in1=rsb[0:ksh, 256:257],
                                           op0=Alu.mult, op1=Alu.add)
            nc.vector.tensor_scalar_mul(csn[0:ksh, :], csn[0:ksh, :], 0.2)
            nc.vector.scalar_tensor_tensor(out=avn[0:ksh, :], in0=av[0:ksh, :],
                                           scalar=4.0, in1=rsb[0:ksh, 0:256],
                                           op0=Alu.mult, op1=Alu.add)
            nc.vector.tensor_scalar_mul(avn[0:ksh, :], avn[0:ksh, :], 0.2)
            den = opool.tile([128, 1], F32)
            nc.vector.tensor_scalar_add(den[0:ksh, :], csn[0:ksh, :], EPS)
            rec = opool.tile([128, 1], F32)
            nc.vector.reciprocal(rec[0:ksh, :], den[0:ksh, :])
            nc.vector.tensor_scalar(out=emn[0:ksh, :], in0=avn[0:ksh, :],
                                    scalar1=rec[0:ksh, :], scalar2=None,
                                    op0=Alu.mult)
            nc.sync.dma_start(csn_d[:].rearrange("(a b) -> a b", b=1),
                              csn[0:ksh, 0:1])
            nc.sync.dma_start(avn_d[:], avn[0:ksh, :])
            nc.sync.dma_start(emn_d[:], emn[0:ksh, :])

    nc.compile()
    return nc


_NC_CACHE = {}


def _get_nc():
    key = (N_CORES, NTILES)
    if key not in _NC_CACHE:
        _NC_CACHE[key] = build_kernel(*key)
    return _NC_CACHE[key]


LAST_RESULTS = None


def kernel(x, embed, cluster_size, embed_avg, _trace=False):
    global LAST_RESULTS
    nc = _get_nc()
    xf = np.ascontiguousarray(np.asarray(x).reshape(-1, D), dtype=np.float32)
    emb = np.ascontiguousarray(embed, np.float32)
    in_maps = []
    for c in range(N_CORES):
        in_maps.append({"x_sh": xf[c * NT:(c + 1) * NT], "embed": emb})
    res = run_bass_kernel_spmd(nc, in_maps, core_ids=list(range(N_CORES)),
                               trace=_trace)
    LAST_RESULTS = res
    outs = res.results
    quant = np.concatenate([r["quant_sh"] for r in outs]).reshape(B, T, D)
    ind = np.concatenate(
        [r["ind_sh"].T.reshape(-1) for r in outs]).reshape(B, T)
    # cross-core reduction of per-core [K, 256] complement partials + EMA:
    # device computed sum_t (1-onehot)*x_h, so esum = colsum - partials;
    # counts are exactly bincount of the returned indices
    comp = np.sum([r["part_sh"] for r in outs], axis=0, dtype=np.float64)
    xh = xf.astype(np.float16).astype(np.float64)
    esum = (xh.sum(0)[None, :] - comp).astype(np.float32)
    counts = np.bincount(ind.reshape(-1), minlength=K).astype(np.float32)
    cs_new = (np.float32(DECAY) * np.asarray(cluster_size, np.float32)
              + np.float32(1.0 - DECAY) * counts)
    av_new = (np.float32(DECAY) * np.asarray(embed_avg, np.float32)
              + np.float32(1.0 - DECAY) * esum)
    em_new = av_new / (cs_new + np.float32(EPS))[:, None]
    return quant, ind.astype(np.int32), cs_new, av_new, em_new


# revision 43
# speedup vs baseline: 1.2438x; 1.0016x over previous
"""EuclideanCodebook (VQ) Trainium2 Bass kernel.

Strategy (data-parallel over 8 NeuronCores, x sharded on tokens):
  per 128-token tile:
    - scores s[t,k] = 2*x.e_k  via fp16 hi/lo split matmuls (3 passes, exact to
      ~2^-22) accumulated in PSUM fp32;
    - fused DVE tensor_tensor_reduce: adj = s - |e_k|^2, m[t] = max_k adj
    - onehot = (adj >= m) on DVE (exact fp32 compare), fp16
    - ind[t] = sum_k onehot*iota  (scalar_tensor_tensor with accum)
    - quantize rows gathered from embed DRAM via indirect DMA
    - segment sums: onehot.T @ [x_h] accumulated in PSUM across all tiles
      (8 k-chunks) + count column via onehot.T @ ones
  epilogue: ReduceScatter(sum) of [1024,257] partials over 8 cores; each core
  EMA-updates its 128-row shard of cluster_size/embed_avg/embed.
Host side only shards/concats and reorders embed_ind.
"""
import numpy as np
from contextlib import ExitStack

import concourse.bass as bass
import concourse.bacc as bacc
import concourse.mybir as mybir
import concourse.tile as tile
from concourse.bass_utils import run_bass_kernel_spmd

dt = mybir.dt
F32 = dt.float32
F16 = dt.float16
I32 = dt.int32
Alu = mybir.AluOpType
Act = mybir.ActivationFunctionType

# problem shapes (hardcoded per contract)
B, T, D, K = 8, 8192, 256, 1024
N_CORES = 8
NT = B * T // N_CORES          # tokens per core (8192)
NTILES = NT // 128             # 64
KC = K // 128                  # 8 k-chunks
KSH = K // N_CORES             # 128 codes per core for EMA shard
DECAY, EPS = 0.8, 1e-5


def build_kernel(n_cores=N_CORES, ntiles=NTILES, use_cc=False):
    nt = ntiles * 128
    ksh = K // n_cores
    nc = bacc.Bacc("TRN2", target_bir_lowering=False, debug=False,
                   num_devices=n_cores)

    x_d = nc.dram_tensor("x_sh", [nt, D], F32, kind="ExternalInput")
    e_d = nc.dram_tensor("embed", [K, D], F32, kind="ExternalInput")
    if use_cc:
        cs_d = nc.dram_tensor("cs_sh", [ksh], F32, kind="ExternalInput")
        av_d = nc.dram_tensor("av_sh", [ksh, D], F32, kind="ExternalInput")

    q_d = nc.dram_tensor("quant_sh", [nt, D], F32, kind="ExternalOutput")
    ind_d = nc.dram_tensor("ind_sh", [128, ntiles], I32, kind="ExternalOutput")
    if use_cc:
        csn_d = nc.dram_tensor("cs_new_sh", [ksh], F32, kind="ExternalOutput")
        avn_d = nc.dram_tensor("av_new_sh", [ksh, D], F32,
                               kind="ExternalOutput")
        emn_d = nc.dram_tensor("em_new_sh", [ksh, D], F32,
                               kind="ExternalOutput")
    else:
        part_d = nc.dram_tensor("part_sh", [K, 256], F32,
                                kind="ExternalOutput")

    with tile.TileContext(nc) as tc, ExitStack() as ctx:
        cpool = ctx.enter_context(tc.tile_pool(name="consts", bufs=1))
        spool = ctx.enter_context(tc.tile_pool(name="setup", bufs=1))
        xpool = ctx.enter_context(tc.tile_pool(name="x", bufs=6))
        wpool = ctx.enter_context(tc.tile_pool(name="work", bufs=4))
        qpool = ctx.enter_context(tc.tile_pool(name="q", bufs=6))
        opool = ctx.enter_context(tc.tile_pool(name="out1", bufs=1))
        ps_s = ctx.enter_context(tc.tile_pool(name="pscore", bufs=4,
                                              space="PSUM"))
        ps_a = ctx.enter_context(tc.tile_pool(name="pacc", bufs=1,
                                              space="PSUM"))
        dpool = ctx.enter_context(tc.tile_pool(name="dram", bufs=1,
                                               space="DRAM"))

        # ---------------- constants / setup ----------------
        embT_h0 = cpool.tile([128, K], F16)   # d 0:128, k, fp16 hi of 2*e^T
        embT_h1 = cpool.tile([128, K], F16)   # d 128:256
        embT_l0 = cpool.tile([128, K], F16)
        embT_l1 = cpool.tile([128, K], F16)
        e2row = cpool.tile([1, K], F32)
        e2full = cpool.tile([128, K], F32)
        iota_row = cpool.tile([1, K], F16)
        iotafull = cpool.tile([128, K], F16)
        ones_r32 = cpool.tile([1, 128], F32)
        ones_r16 = cpool.tile([1, 128], F16)
        ident32 = cpool.tile([128, 128], F32)
        ident16 = cpool.tile([128, 128], F16)
        ones_c32 = cpool.tile([128, 1], F32)
        ones_c16 = cpool.tile([128, 1], F16)
        indacc = opool.tile([128, ntiles], I32)

        # persistent PSUM: scatter accumulator (4 banks); counts are derived
        # host-side from the embed_ind output (bincount)
        pscat = ps_a.tile([128, 2048], F32)

        ones128 = spool.tile([128, 128], F32)
        nc.vector.memset(ones128[:], 1.0)
        nc.gpsimd.affine_select(ident32[:], ones128[:], pattern=[[-1, 128]],
                                base=0, channel_multiplier=1,
                                compare_op=Alu.is_equal, fill=0.0)
        nc.vector.tensor_copy(ident16[:], ident32[:])
        nc.vector.memset(ones_c32[:], 1.0)
        nc.vector.memset(ones_c16[:], 1.0)

        iota_i = spool.tile([1, K], I32)
        nc.gpsimd.iota(iota_i[:], pattern=[[1, K]], base=0,
                       channel_multiplier=0)
        nc.vector.tensor_copy(iota_row[:], iota_i[:])
        nc.vector.memset(ones_r32[:], 1.0)
        nc.vector.memset(ones_r16[:], 1.0)
        # broadcast iota to all partitions: ones[1,128].T @ iota_row
        for h in range(2):
            ks = slice(h * 512, (h + 1) * 512)
            pb = ps_s.tile([128, 512], F32, name="pb", tag="ps")
            nc.tensor.matmul(pb[:], ones_r16[:], iota_row[0:1, ks])
            nc.vector.tensor_copy(iotafull[:, ks], pb[:])

        # embed transpose + split + e2
        et = spool.tile([128, D], F32)
        t2 = spool.tile([128, D], F32)
        sq = spool.tile([128, 128], F32)
        for c in range(KC):
            et_ = spool.tile([128, D], F32, name=f"et{c}", tag="et")
            nc.sync.dma_start(et_[:], e_d[c * 128:(c + 1) * 128, :])
            # transpose both d-halves into a score-pool bank (exact fp32 e^T);
            # single psum group per chunk (2nd start would re-zero the bank)
            pet = ps_s.tile([128, 512], F32, name="pet", tag="ps")
            nc.tensor.matmul(pet[:, 0:128], et_[:, 0:128], ident32[:],
                             is_transpose=True, start=True, stop=False)
            nc.tensor.matmul(pet[:, 128:256], et_[:, 128:256], ident32[:],
                             is_transpose=True, start=False, stop=True)
            t2_ = spool.tile([128, D], F32, name=f"t2{c}", tag="t2")
            nc.scalar.mul(t2_[:], pet[:, 0:256], 2.0)
            kk = slice(c * 128, (c + 1) * 128)
            nc.scalar.copy(embT_h0[:, kk], t2_[:, 0:128])
            nc.scalar.copy(embT_h1[:, kk], t2_[:, 128:256])
            nc.vector.tensor_tensor(out=embT_l0[:, kk], in0=t2_[:, 0:128],
                                    in1=embT_h0[:, kk], op=Alu.subtract)
            nc.vector.tensor_tensor(out=embT_l1[:, kk], in0=t2_[:, 128:256],
                                    in1=embT_h1[:, kk], op=Alu.subtract)
            # e2 contribution: sum_d e^2 = ones.T @ (eT*eT)
            # one psum group per 512-wide bank (chunks 4c..4c+3)
            sq0 = spool.tile([128, 128], F32, name=f"sq0{c}", tag="sq0")
            sq1 = spool.tile([128, 128], F32, name=f"sq1{c}", tag="sq1")
            nc.scalar.square(sq0[:], pet[:, 0:128])
            nc.scalar.square(sq1[:], pet[:, 128:256])
            nc.tensor.matmul(pscat[0:1, kk], ones_c32[:], sq0[:],
                             start=(c % 4 == 0), stop=False)
            nc.tensor.matmul(pscat[0:1, kk], ones_c32[:], sq1[:],
                             start=False, stop=(c % 4 == 3))
        nc.vector.tensor_copy(e2row[0:1, :], pscat[0:1, 0:K])
        # broadcast e2 to all partitions (fp32 matmul, 512-wide chunks)
        for h in range(2):
            ks = slice(h * 512, (h + 1) * 512)
            pb2 = ps_s.tile([128, 512], F32, name="pb2", tag="ps")
            nc.tensor.matmul(pb2[:], ones_r32[:], e2row[0:1, ks])
            nc.vector.tensor_copy(e2full[:, ks], pb2[:])

        # ---------------- main loop ----------------
        # the scatter matmuls for tile t are emitted during iteration t+1 so
        # the PE never stalls waiting for tile t's onehot (ACT) to land
        def emit_scatter(t, oh, xaug):
            for c in range(KC):
                kk = slice(c * 128, (c + 1) * 128)
                nc.tensor.matmul(pscat[:, c * 256:(c + 1) * 256],
                                 oh[:, kk], xaug[:, 0:256],
                                 start=(t == 0 and c % 2 == 0),
                                 stop=(t == ntiles - 1 and c % 2 == 1))

        prev = None
        for t in range(ntiles):
            rows = slice(t * 128, (t + 1) * 128)
            xt = xpool.tile([128, D], F32, name="xt", tag="xt")
            nc.sync.dma_start(xt[:], x_d[rows, :])
            # fp16 split
            xaug = xpool.tile([128, 256], F16, name="xaug", tag="xaug")
            nc.scalar.copy(xaug[:, 0:256], xt[:])
            xl = xpool.tile([128, D], F16, name="xl", tag="xl")
            nc.vector.tensor_tensor(out=xl[:], in0=xt[:], in1=xaug[:, 0:256],
                                    op=Alu.subtract)
            # transposes (fp16) into a rotating score-pool bank viewed as f16:
            # [0:256]f16 = xh^T, [256:512]f16 = xl^T. One psum group of 4.
            pxt = ps_s.tile([128, 512], F32, name="pxt", tag="ps")
            pf16 = pxt[:].bitcast(F16)
            nc.tensor.matmul(pf16[:, 0:128], xaug[:, 0:128], ident16[:],
                             is_transpose=True, start=True, stop=False)
            nc.tensor.matmul(pf16[:, 128:256], xaug[:, 128:256], ident16[:],
                             is_transpose=True, start=False, stop=False)
            nc.tensor.matmul(pf16[:, 256:384], xl[:, 0:128], ident16[:],
                             is_transpose=True, start=False, stop=False)
            nc.tensor.matmul(pf16[:, 384:512], xl[:, 128:256], ident16[:],
                             is_transpose=True, start=False, stop=True)
            xth = xpool.tile([128, D], F16, name="xth", tag="xth")
            xtl = xpool.tile([128, D], F16, name="xtl", tag="xtl")
            nc.scalar.copy(xth[:], pf16[:, 0:256])
            nc.scalar.copy(xtl[:], pf16[:, 256:512])

            adj = wpool.tile([128, K], F32, name="adj", tag="adj")
            m01 = wpool.tile([128, 2], F32, name="m01", tag="m01")
            # 12 accumulating matmuls ordered for stationary-weight reuse:
            # each weight (xth/xtl d-chunk) serves its rhs over both k-halves
            plan = [(xth, 0, embT_h0), (xth, 1, embT_h1),
                    (xth, 0, embT_l0), (xth, 1, embT_l1),
                    (xtl, 0, embT_h0), (xtl, 1, embT_h1)]
            for h in range(2):
                ks = slice(h * 512, (h + 1) * 512)
                ps = ps_s.tile([128, 512], F32, name="ps", tag="ps")
                for i, (w, dc, rhs) in enumerate(plan):
                    nc.tensor.matmul(ps[:], w[:, dc * 128:(dc + 1) * 128],
                                     rhs[:, ks], start=(i == 0),
                                     stop=(i == len(plan) - 1))
                nc.vector.tensor_tensor(out=adj[:, ks], in0=ps[:],
                                        in1=e2full[:, ks], op=Alu.subtract)
                nc.vector.tensor_reduce(out=m01[:, h:h + 1], in_=adj[:, ks],
                                        axis=mybir.AxisListType.X, op=Alu.max)
            if prev is not None:
                emit_scatter(t - 1, *prev)
            m = wpool.tile([128, 1], F32, name="m", tag="m")
            nc.vector.tensor_tensor(out=m[:], in0=m01[:, 0:1],
                                    in1=m01[:, 1:2], op=Alu.max)
            # complement onehot on ACT: Sign(m - adj) = {0 hit, +1 miss};
            # scatter then yields colsum - seg, fixed up on the host.
            oh = wpool.tile([128, K], F16, name="oh", tag="oh")
            nc.scalar.activation(oh[:], adj[:], Act.Sign, bias=m[:],
                                 scale=-1.0)
            # index extraction: hits are 0 -> (oh <= 0.5) * iota
            indf = wpool.tile([128, 1], F32, name="indf", tag="indf")
            ohs = wpool.tile([128, K], F16, name="ohs", tag="ohs")
            nc.vector.scalar_tensor_tensor(
                out=ohs[:], in0=oh[:], scalar=0.5,
                in1=iotafull[:],
                op0=Alu.is_le, op1=Alu.mult, accum_out=indf[:])
            indi = wpool.tile([128, 1], I32, name="indi", tag="indi")
            nc.vector.tensor_copy(indi[:], indf[:])
            nc.vector.tensor_copy(indacc[:, t:t + 1], indi[:])
            # quantize gather: embed rows by index (DRAM -> SBUF -> DRAM)
            q = qpool.tile([128, D], F32, name="q", tag="q")
            nc.gpsimd.indirect_dma_start(
                out=q[:], out_offset=None, in_=e_d[:],
                in_offset=bass.IndirectOffsetOnAxis(ap=indi[:], axis=0))
            nc.sync.dma_start(q_d[rows, :], q[:])
            prev = (oh, xaug)

        emit_scatter(ntiles - 1, *prev)
        nc.sync.dma_start(ind_d[:], indacc[:])

        # ---------------- epilogue: reduce + EMA ----------------
        if not use_cc:
            # flush per-core partial sums; host reduces across cores
            for c in range(KC):
                st = wpool.tile([128, 256], F32, name="st", tag="st")
                nc.scalar.copy(st[:], pscat[:, c * 256:(c + 1) * 256])
                nc.sync.dma_start(part_d[c * 128:(c + 1) * 128, :], st[:])
        else:
            # NOTE: collective_compute hangs under the axon/PJRT path in this
            # container; this branch is kept for native-NRT environments.
            rs_in = dpool.tile([K, 257], F32)
            rs_out = dpool.tile([K // n_cores, 257], F32)
            for c in range(KC):
                st = wpool.tile([128, 257], F32, name="st", tag="st")
                nc.scalar.copy(st[:, 0:256], pscat[:, c * 256:(c + 1) * 256])
                nc.vector.memset(st[:, 256:257], 0.0)
                nc.sync.dma_start(rs_in[c * 128:(c + 1) * 128, :], st[:])
            nc.gpsimd.collective_compute(
                "ReduceScatter", Alu.add,
                replica_groups=[list(range(n_cores))],
                ins=[rs_in.opt()], outs=[rs_out.opt()])

            rsb = opool.tile([128, 257], F32)
            nc.sync.dma_start(rsb[0:ksh, :], rs_out[:])
            cs = opool.tile([128, 1], F32)
            av = opool.tile([128, D], F32)
            nc.sync.dma_start(cs[0:ksh, 0:1],
                              cs_d[:].rearrange("(a b) -> a b", b=1))
            nc.sync.dma_start(av[0:ksh, :], av_d[:])
            csn = opool.tile([128, 1], F32)
            avn = opool.tile([128, D], F32)
            emn = opool.tile([128, D], F32)
            # new = (old*4 + seg) * 0.2  == old*0.8 + seg*0.2
            nc.vector.scalar_tensor_tensor(out=csn[0:ksh, :], in0=cs[0:ksh, :],
                                           scalar=4.0, in1=rsb[0:ksh, 256:257],
                                           op0=Alu.mult, op1=Alu.add)
            nc.vector.tensor_scalar_mul(csn[0:ksh, :], csn[0:ksh, :], 0.2)
            nc.vector.scalar_tensor_tensor(out=avn[0:ksh, :], in0=av[0:ksh, :],
                                           scalar=4.0, in1=rsb[0:ksh, 0:256],
                                           op0=Alu.mult, op1=Alu.add)
            nc.vector.tensor_scalar_mul(avn[0:ksh, :], avn[0:ksh, :], 0.2)
            den = opool.tile([128, 1], F32)
            nc.vector.tensor_scalar_add(den[0:ksh, :], csn[0:ksh, :], EPS)
            rec = opool.tile([128, 1], F32)
            nc.vector.reciprocal(rec[0:ksh, :], den[0:ksh, :])
            nc.vector.tensor_scalar(out=emn[0:ksh, :], in0=avn[0:ksh, :],
                                    scalar1=rec[0:ksh, :], scalar2=None,
                                    op0=Alu.mult)
            nc.sync.dma_start(csn_d[:].rearrange("(a b) -> a b", b=1),
                              csn[0:ksh, 0:1])
            nc.sync.dma_start(avn_d[:], avn[0:ksh, :])
            nc.sync.dma_start(emn_d[:], emn[0:ksh, :])

    nc.compile()
    return nc


_NC_CACHE = {}


def _get_nc():
    key = (N_CORES, NTILES)
    if key not in _NC_CACHE:
        _NC_CACHE[key] = build_kernel(*key)
    return _NC_CACHE[key]


LAST_RESULTS = None


def kernel(x, embed, cluster_size, embed_avg, _trace=False):
    global LAST_RESULTS
    nc = _get_nc()
    xf = np.ascontiguousarray(np.asarray(x).reshape(-1, D), dtype=np.float32)
    emb = np.ascontiguousarray(embed, np.float32)
    in_maps = []
    for c in range(N_CORES):
        in_maps.append({"x_sh": xf[c * NT:(c + 1) * NT], "embed": emb})
    res = run_bass_kernel_spmd(nc, in_maps, core_ids=list(range(N_CORES)),
                               trace=_trace)
    LAST_RESULTS = res
    outs = res.results
    quant = np.concatenate([r["quant_sh"] for r in outs]).reshape(B, T, D)
    ind = np.concatenate(
        [r["ind_sh"].T.reshape(-1) for r in outs]).reshape(B, T)
    # cross-core reduction of per-core [K, 256] complement partials + EMA:
    # device computed sum_t (1-onehot)*x_h, so esum = colsum - partials;
    # counts are exactly bincount of the returned indices
    comp = np.sum([r["part_sh"] for r in outs], axis=0, dtype=np.float64)
    xh = xf.astype(np.float16).astype(np.float64)
    esum = (xh.sum(0)[None, :] - comp).astype(np.float32)
    counts = np.bincount(ind.reshape(-1), minlength=K).astype(np.float32)
    cs_new = (np.float32(DECAY) * np.asarray(cluster_size, np.float32)
              + np.float32(1.0 - DECAY) * counts)
    av_new = (np.float32(DECAY) * np.asarray(embed_avg, np.float32)
              + np.float32(1.0 - DECAY) * esum)
    em_new = av_new / (cs_new + np.float32(EPS))[:, None]
    return quant, ind.astype(np.int32), cs_new, av_new, em_new


# revision 44
# speedup vs baseline: 1.2694x; 1.0205x over previous
"""EuclideanCodebook (VQ) Trainium2 Bass kernel.

Strategy (data-parallel over 8 NeuronCores, x sharded on tokens):
  per 128-token tile:
    - scores s[t,k] = 2*x.e_k  via fp16 hi/lo split matmuls (3 passes, exact to
      ~2^-22) accumulated in PSUM fp32;
    - fused DVE tensor_tensor_reduce: adj = s - |e_k|^2, m[t] = max_k adj
    - onehot = (adj >= m) on DVE (exact fp32 compare), fp16
    - ind[t] = sum_k onehot*iota  (scalar_tensor_tensor with accum)
    - quantize rows gathered from embed DRAM via indirect DMA
    - segment sums: onehot.T @ [x_h] accumulated in PSUM across all tiles
      (8 k-chunks) + count column via onehot.T @ ones
  epilogue: ReduceScatter(sum) of [1024,257] partials over 8 cores; each core
  EMA-updates its 128-row shard of cluster_size/embed_avg/embed.
Host side only shards/concats and reorders embed_ind.
"""
import numpy as np
from contextlib import ExitStack

import concourse.bass as bass
import concourse.bacc as bacc
import concourse.mybir as mybir
import concourse.tile as tile
from concourse.bass_utils import run_bass_kernel_spmd

dt = mybir.dt
F32 = dt.float32
F16 = dt.float16
I32 = dt.int32
Alu = mybir.AluOpType
Act = mybir.ActivationFunctionType

# problem shapes (hardcoded per contract)
B, T, D, K = 8, 8192, 256, 1024
N_CORES = 8
NT = B * T // N_CORES          # tokens per core (8192)
NTILES = NT // 128             # 64
KC = K // 128                  # 8 k-chunks
KSH = K // N_CORES             # 128 codes per core for EMA shard
DECAY, EPS = 0.8, 1e-5


def build_kernel(n_cores=N_CORES, ntiles=NTILES, use_cc=False):
    nt = ntiles * 128
    ksh = K // n_cores
    nc = bacc.Bacc("TRN2", target_bir_lowering=False, debug=False,
                   num_devices=n_cores)

    x_d = nc.dram_tensor("x_sh", [nt, D], F32, kind="ExternalInput")
    e_d = nc.dram_tensor("embed", [K, D], F32, kind="ExternalInput")
    if use_cc:
        cs_d = nc.dram_tensor("cs_sh", [ksh], F32, kind="ExternalInput")
        av_d = nc.dram_tensor("av_sh", [ksh, D], F32, kind="ExternalInput")

    q_d = nc.dram_tensor("quant_sh", [nt, D], F32, kind="ExternalOutput")
    ind_d = nc.dram_tensor("ind_sh", [128, ntiles], I32, kind="ExternalOutput")
    if use_cc:
        csn_d = nc.dram_tensor("cs_new_sh", [ksh], F32, kind="ExternalOutput")
        avn_d = nc.dram_tensor("av_new_sh", [ksh, D], F32,
                               kind="ExternalOutput")
        emn_d = nc.dram_tensor("em_new_sh", [ksh, D], F32,
                               kind="ExternalOutput")
    else:
        part_d = nc.dram_tensor("part_sh", [K, 256], F32,
                                kind="ExternalOutput")

    with tile.TileContext(nc) as tc, ExitStack() as ctx:
        cpool = ctx.enter_context(tc.tile_pool(name="consts", bufs=1))
        spool = ctx.enter_context(tc.tile_pool(name="setup", bufs=1))
        xpool = ctx.enter_context(tc.tile_pool(name="x", bufs=6))
        wpool = ctx.enter_context(tc.tile_pool(name="work", bufs=4))
        qpool = ctx.enter_context(tc.tile_pool(name="q", bufs=6))
        opool = ctx.enter_context(tc.tile_pool(name="out1", bufs=1))
        ps_s = ctx.enter_context(tc.tile_pool(name="pscore", bufs=4,
                                              space="PSUM"))
        ps_a = ctx.enter_context(tc.tile_pool(name="pacc", bufs=1,
                                              space="PSUM"))
        dpool = ctx.enter_context(tc.tile_pool(name="dram", bufs=1,
                                               space="DRAM"))

        # ---------------- constants / setup ----------------
        embT_h0 = cpool.tile([128, K], F16)   # d 0:128, k, fp16 hi of 2*e^T
        embT_h1 = cpool.tile([128, K], F16)   # d 128:256
        embT_l0 = cpool.tile([128, K], F16)
        embT_l1 = cpool.tile([128, K], F16)
        e2row = cpool.tile([1, K], F32)
        e2full = cpool.tile([128, K], F32)
        iota_row = cpool.tile([1, K], F16)
        iotafull = cpool.tile([128, K], F16)
        ones_r32 = cpool.tile([1, 128], F32)
        ones_r16 = cpool.tile([1, 128], F16)
        ident32 = cpool.tile([128, 128], F32)
        ident16 = cpool.tile([128, 128], F16)
        ones_c32 = cpool.tile([128, 1], F32)
        ones_c16 = cpool.tile([128, 1], F16)
        indacc = opool.tile([128, ntiles], I32)

        # persistent PSUM: scatter accumulator (4 banks); counts are derived
        # host-side from the embed_ind output (bincount)
        pscat = ps_a.tile([128, 2048], F32)

        ones128 = spool.tile([128, 128], F32)
        nc.vector.memset(ones128[:], 1.0)
        nc.gpsimd.affine_select(ident32[:], ones128[:], pattern=[[-1, 128]],
                                base=0, channel_multiplier=1,
                                compare_op=Alu.is_equal, fill=0.0)
        nc.vector.tensor_copy(ident16[:], ident32[:])
        nc.vector.memset(ones_c32[:], 1.0)
        nc.vector.memset(ones_c16[:], 1.0)

        iota_i = spool.tile([1, K], I32)
        nc.gpsimd.iota(iota_i[:], pattern=[[1, K]], base=0,
                       channel_multiplier=0)
        nc.vector.tensor_copy(iota_row[:], iota_i[:])
        nc.vector.memset(ones_r32[:], 1.0)
        nc.vector.memset(ones_r16[:], 1.0)
        # broadcast iota to all partitions: ones[1,128].T @ iota_row
        for h in range(2):
            ks = slice(h * 512, (h + 1) * 512)
            pb = ps_s.tile([128, 512], F32, name="pb", tag="ps")
            nc.tensor.matmul(pb[:], ones_r16[:], iota_row[0:1, ks])
            nc.vector.tensor_copy(iotafull[:, ks], pb[:])

        # embed transpose + split + e2
        et = spool.tile([128, D], F32)
        t2 = spool.tile([128, D], F32)
        sq = spool.tile([128, 128], F32)
        for c in range(KC):
            et_ = spool.tile([128, D], F32, name=f"et{c}", tag="et")
            nc.sync.dma_start(et_[:], e_d[c * 128:(c + 1) * 128, :])
            # transpose both d-halves into a score-pool bank (exact fp32 e^T);
            # single psum group per chunk (2nd start would re-zero the bank)
            pet = ps_s.tile([128, 512], F32, name="pet", tag="ps")
            nc.tensor.matmul(pet[:, 0:128], et_[:, 0:128], ident32[:],
                             is_transpose=True, start=True, stop=False)
            nc.tensor.matmul(pet[:, 128:256], et_[:, 128:256], ident32[:],
                             is_transpose=True, start=False, stop=True)
            t2_ = spool.tile([128, D], F32, name=f"t2{c}", tag="t2")
            nc.scalar.mul(t2_[:], pet[:, 0:256], 2.0)
            kk = slice(c * 128, (c + 1) * 128)
            nc.scalar.copy(embT_h0[:, kk], t2_[:, 0:128])
            nc.scalar.copy(embT_h1[:, kk], t2_[:, 128:256])
            nc.vector.tensor_tensor(out=embT_l0[:, kk], in0=t2_[:, 0:128],
                                    in1=embT_h0[:, kk], op=Alu.subtract)
            nc.vector.tensor_tensor(out=embT_l1[:, kk], in0=t2_[:, 128:256],
                                    in1=embT_h1[:, kk], op=Alu.subtract)
            # e2 contribution: sum_d e^2 = ones.T @ (eT*eT)
            # one psum group per 512-wide bank (chunks 4c..4c+3)
            sq0 = spool.tile([128, 128], F32, name=f"sq0{c}", tag="sq0")
            sq1 = spool.tile([128, 128], F32, name=f"sq1{c}", tag="sq1")
            nc.scalar.square(sq0[:], pet[:, 0:128])
            nc.scalar.square(sq1[:], pet[:, 128:256])
            nc.tensor.matmul(pscat[0:1, kk], ones_c32[:], sq0[:],
                             start=(c % 4 == 0), stop=False)
            nc.tensor.matmul(pscat[0:1, kk], ones_c32[:], sq1[:],
                             start=False, stop=(c % 4 == 3))
        nc.vector.tensor_copy(e2row[0:1, :], pscat[0:1, 0:K])
        # broadcast e2 to all partitions (fp32 matmul, 512-wide chunks)
        for h in range(2):
            ks = slice(h * 512, (h + 1) * 512)
            pb2 = ps_s.tile([128, 512], F32, name="pb2", tag="ps")
            nc.tensor.matmul(pb2[:], ones_r32[:], e2row[0:1, ks])
            nc.vector.tensor_copy(e2full[:, ks], pb2[:])

        # ---------------- main loop ----------------
        # the scatter matmuls for tile t are emitted during iteration t+1 so
        # the PE never stalls waiting for tile t's onehot (ACT) to land
        def emit_scatter(t, oh, xaug):
            for c in range(KC):
                kk = slice(c * 128, (c + 1) * 128)
                nc.tensor.matmul(pscat[:, c * 256:(c + 1) * 256],
                                 oh[:, kk], xaug[:, 0:256],
                                 start=(t == 0 and c % 2 == 0),
                                 stop=(t == ntiles - 1 and c % 2 == 1))

        prev = None
        for t in range(ntiles):
            rows = slice(t * 128, (t + 1) * 128)
            xt = xpool.tile([128, D], F32, name="xt", tag="xt")
            nc.sync.dma_start(xt[:], x_d[rows, :])
            # fp16 split
            xaug = xpool.tile([128, 256], F16, name="xaug", tag="xaug")
            nc.gpsimd.tensor_copy(xaug[:, 0:256], xt[:])
            xl = xpool.tile([128, D], F16, name="xl", tag="xl")
            nc.vector.tensor_tensor(out=xl[:], in0=xt[:], in1=xaug[:, 0:256],
                                    op=Alu.subtract)
            # transposes (fp16) into a rotating score-pool bank viewed as f16:
            # [0:256]f16 = xh^T, [256:512]f16 = xl^T. One psum group of 4.
            pxt = ps_s.tile([128, 512], F32, name="pxt", tag="ps")
            pf16 = pxt[:].bitcast(F16)
            nc.tensor.matmul(pf16[:, 0:128], xaug[:, 0:128], ident16[:],
                             is_transpose=True, start=True, stop=False)
            nc.tensor.matmul(pf16[:, 128:256], xaug[:, 128:256], ident16[:],
                             is_transpose=True, start=False, stop=False)
            nc.tensor.matmul(pf16[:, 256:384], xl[:, 0:128], ident16[:],
                             is_transpose=True, start=False, stop=False)
            nc.tensor.matmul(pf16[:, 384:512], xl[:, 128:256], ident16[:],
                             is_transpose=True, start=False, stop=True)
            xth = xpool.tile([128, D], F16, name="xth", tag="xth")
            xtl = xpool.tile([128, D], F16, name="xtl", tag="xtl")
            nc.scalar.copy(xth[:], pf16[:, 0:256])
            nc.scalar.copy(xtl[:], pf16[:, 256:512])

            adj = wpool.tile([128, K], F32, name="adj", tag="adj")
            m01 = wpool.tile([128, 2], F32, name="m01", tag="m01")
            # 12 accumulating matmuls ordered for stationary-weight reuse:
            # each weight (xth/xtl d-chunk) serves its rhs over both k-halves
            plan = [(xth, 0, embT_h0), (xth, 1, embT_h1),
                    (xth, 0, embT_l0), (xth, 1, embT_l1),
                    (xtl, 0, embT_h0), (xtl, 1, embT_h1)]
            for h in range(2):
                ks = slice(h * 512, (h + 1) * 512)
                ps = ps_s.tile([128, 512], F32, name="ps", tag="ps")
                for i, (w, dc, rhs) in enumerate(plan):
                    nc.tensor.matmul(ps[:], w[:, dc * 128:(dc + 1) * 128],
                                     rhs[:, ks], start=(i == 0),
                                     stop=(i == len(plan) - 1))
                nc.vector.tensor_tensor(out=adj[:, ks], in0=ps[:],
                                        in1=e2full[:, ks], op=Alu.subtract)
                nc.vector.tensor_reduce(out=m01[:, h:h + 1], in_=adj[:, ks],
                                        axis=mybir.AxisListType.X, op=Alu.max)
            if prev is not None:
                emit_scatter(t - 1, *prev)
            m = wpool.tile([128, 1], F32, name="m", tag="m")
            nc.vector.tensor_tensor(out=m[:], in0=m01[:, 0:1],
                                    in1=m01[:, 1:2], op=Alu.max)
            # complement onehot on ACT: Sign(m - adj) = {0 hit, +1 miss};
            # scatter then yields colsum - seg, fixed up on the host.
            oh = wpool.tile([128, K], F16, name="oh", tag="oh")
            nc.scalar.activation(oh[:], adj[:], Act.Sign, bias=m[:],
                                 scale=-1.0)
            # index extraction: hits are 0 -> (oh <= 0.5) * iota
            indf = wpool.tile([128, 1], F32, name="indf", tag="indf")
            ohs = wpool.tile([128, K], F16, name="ohs", tag="ohs")
            nc.vector.scalar_tensor_tensor(
                out=ohs[:], in0=oh[:], scalar=0.5,
                in1=iotafull[:],
                op0=Alu.is_le, op1=Alu.mult, accum_out=indf[:])
            indi = wpool.tile([128, 1], I32, name="indi", tag="indi")
            nc.vector.tensor_copy(indi[:], indf[:])
            nc.vector.tensor_copy(indacc[:, t:t + 1], indi[:])
            # quantize gather: embed rows by index (DRAM -> SBUF -> DRAM)
            q = qpool.tile([128, D], F32, name="q", tag="q")
            nc.gpsimd.indirect_dma_start(
                out=q[:], out_offset=None, in_=e_d[:],
                in_offset=bass.IndirectOffsetOnAxis(ap=indi[:], axis=0))
            nc.sync.dma_start(q_d[rows, :], q[:])
            prev = (oh, xaug)

        emit_scatter(ntiles - 1, *prev)
        nc.sync.dma_start(ind_d[:], indacc[:])

        # ---------------- epilogue: reduce + EMA ----------------
        if not use_cc:
            # flush per-core partial sums; host reduces across cores
            for c in range(KC):
                st = wpool.tile([128, 256], F32, name="st", tag="st")
                nc.scalar.copy(st[:], pscat[:, c * 256:(c + 1) * 256])
                nc.sync.dma_start(part_d[c * 128:(c + 1) * 128, :], st[:])
        else:
            # NOTE: collective_compute hangs under the axon/PJRT path in this
            # container; this branch is kept for native-NRT environments.
            rs_in = dpool.tile([K, 257], F32)
            rs_out = dpool.tile([K // n_cores, 257], F32)
            for c in range(KC):
                st = wpool.tile([128, 257], F32, name="st", tag="st")
                nc.scalar.copy(st[:, 0:256], pscat[:, c * 256:(c + 1) * 256])
                nc.vector.memset(st[:, 256:257], 0.0)
                nc.sync.dma_start(rs_in[c * 128:(c + 1) * 128, :], st[:])
            nc.gpsimd.collective_compute(
                "ReduceScatter", Alu.add,
                replica_groups=[list(range(n_cores))],
                ins=[rs_in.opt()], outs=[rs_out.opt()])

            rsb = opool.tile([128, 257], F32)
            nc.sync.dma_start(rsb[0:ksh, :], rs_out[:])
            cs = opool.tile([128, 1], F32)
            av = opool.tile([128, D], F32)
            nc.sync.dma_start(cs[0:ksh, 0:1],
                              cs_d[:].rearrange("(a b) -> a b", b=1))
            nc.sync.dma_start(av[0:ksh, :], av_d[:])
            csn = opool.tile([128, 1], F32)
            avn = opool.tile([128, D], F32)
            emn = opool.tile([128, D], F32)
            # new = (old*4 + seg) * 0.2  == old*0.8 + seg*0.2
            nc.vector.scalar_tensor_tensor(out=csn[0:ksh, :], in0=cs[0:ksh, :],
                                           scalar=4.0, in1=rsb[0:ksh, 256:257],
                                           op0=Alu.mult, op1=Alu.add)
            nc.vector.tensor_scalar_mul(csn[0:ksh, :], csn[0:ksh, :], 0.2)
            nc.vector.scalar_tensor_tensor(out=avn[0:ksh, :], in0=av[0:ksh, :],
                                           scalar=4.0, in1=rsb[0:ksh, 0:256],
                                           op0=Alu.mult, op1=Alu.add)
            nc.vector.tensor_scalar_mul(avn[0:ksh, :], avn[0:ksh, :], 0.2)
            den = opool.tile([128, 1], F32)
            nc.vector.tensor_scalar_add(den[0:ksh, :], csn[0:ksh, :], EPS)
            rec = opool.tile([128, 1], F32)
            nc.vector.reciprocal(rec[0:ksh, :], den[0:ksh, :])
            nc.vector.tensor_scalar(out=emn[0:ksh, :], in0=avn[0:ksh, :],
                                    scalar1=rec[0:ksh, :], scalar2=None,
                                    op0=Alu.mult)
            nc.sync.dma_start(csn_d[:].rearrange("(a b) -> a b", b=1),
                              csn[0:ksh, 0:1])
            nc.sync.dma_start(avn_d[:], avn[0:ksh, :])
            nc.sync.dma_start(emn_d[:], emn[0:ksh, :])

    nc.compile()
    return nc


_NC_CACHE = {}


def _get_nc():
    key = (N_CORES, NTILES)
    if key not in _NC_CACHE:
        _NC_CACHE[key] = build_kernel(*key)
    return _NC_CACHE[key]


LAST_RESULTS = None


def kernel(x, embed, cluster_size, embed_avg, _trace=False):
    global LAST_RESULTS
    nc = _get_nc()
    xf = np.ascontiguousarray(np.asarray(x).reshape(-1, D), dtype=np.float32)
    emb = np.ascontiguousarray(embed, np.float32)
    in_maps = []
    for c in range(N_CORES):
        in_maps.append({"x_sh": xf[c * NT:(c + 1) * NT], "embed": emb})
    res = run_bass_kernel_spmd(nc, in_maps, core_ids=list(range(N_CORES)),
                               trace=_trace)
    LAST_RESULTS = res
    outs = res.results
    quant = np.concatenate([r["quant_sh"] for r in outs]).reshape(B, T, D)
    ind = np.concatenate(
        [r["ind_sh"].T.reshape(-1) for r in outs]).reshape(B, T)
    # cross-core reduction of per-core [K, 256] complement partials + EMA:
    # device computed sum_t (1-onehot)*x_h, so esum = colsum - partials;
    # counts are exactly bincount of the returned indices
    comp = np.sum([r["part_sh"] for r in outs], axis=0, dtype=np.float64)
    xh = xf.astype(np.float16).astype(np.float64)
    esum = (xh.sum(0)[None, :] - comp).astype(np.float32)
    counts = np.bincount(ind.reshape(-1), minlength=K).astype(np.float32)
    cs_new = (np.float32(DECAY) * np.asarray(cluster_size, np.float32)
              + np.float32(1.0 - DECAY) * counts)
    av_new = (np.float32(DECAY) * np.asarray(embed_avg, np.float32)
              + np.float32(1.0 - DECAY) * esum)
    em_new = av_new / (cs_new + np.float32(EPS))[:, None]
    return quant, ind.astype(np.int32), cs_new, av_new, em_new
